# revision 1
# baseline (speedup 1.0000x reference)
"""Trainium2 Bass kernel for bucketed causal linear self-attention.

Model (B=4, T=4096, DIM=1024, H=16 heads, E=64, BUCKET=64):
  q,k,v = x@Wq, x@Wk, x@Wv ; q softmaxed over head-dim, k -> elu(k)+1
  per-bucket context C_u = cumsum_u(k_bu^T v_bu), normalized by cumsum of
  key-sums, shifted one bucket; attn_bu = q_bu @ C_{u-1}; out = attn@Wo + bo.

Sharding over 8 cores: core c -> batch c//2, head-group c%2 (8 heads = 512
feats). q/k/v column-sharded by head, Wo row-sharded; host sums the two
partial outputs per batch (all-reduce on host) and adds bo.

Per-core math folds:
  softmax: q'' = exp(q) * (1/sum exp(q)) * (1/(ksum_prefix+eps)); bucket-0
  attn is exactly zero (blindspot) via zeroed scale column.
  ksum rides as a 65th ones-column through the context matmul; its bucket
  cumsum uses the DVE tensor_tensor_scan.
"""

import sys
import numpy as np
import ml_dtypes

sys.path.insert(0, "/opt/trn_rl_repo")

B, T, DIM, H, BUCKET = 4, 4096, 1024, 16, 64
E = 64           # head dim
HC = 8           # heads per core
F = HC * E       # per-core feature width = 512
CH = 512         # tokens per chunk
UC = CH // BUCKET  # buckets per chunk = 8
PAIRS = HC // 2  # head pairs = 4
KT = DIM // 128  # contraction tiles = 8
EPS = 1e-6

_NC_CACHE = {}


def build_nc(n_chunks=T // CH):
    import concourse.bass as bass
    import concourse.mybir as mybir
    from concourse import bacc
    from concourse.tile import TileContext

    BF16 = mybir.dt.bfloat16
    F32 = mybir.dt.float32
    AF = mybir.ActivationFunctionType
    OP = mybir.AluOpType

    Tt = n_chunks * CH

    nc = bacc.Bacc("TRN2", target_bir_lowering=False, debug=False, num_devices=8)
    x = nc.dram_tensor("x", [Tt, DIM], BF16, kind="ExternalInput").ap()
    wq = nc.dram_tensor("wq", [DIM, F], BF16, kind="ExternalInput").ap()
    wk = nc.dram_tensor("wk", [DIM, F], BF16, kind="ExternalInput").ap()
    wv = nc.dram_tensor("wv", [DIM, F], BF16, kind="ExternalInput").ap()
    wo = nc.dram_tensor("wo", [F, DIM], BF16, kind="ExternalInput").ap()
    orp = nc.dram_tensor("orp", [2, 128], BF16, kind="ExternalInput").ap()
    out = nc.dram_tensor("out", [Tt, DIM], F32, kind="ExternalOutput").ap()

    with TileContext(nc) as tc:
        with tc.tile_pool(name="const", bufs=1) as constp, \
             tc.tile_pool(name="xt", bufs=2) as xtp, \
             tc.tile_pool(name="act", bufs=2) as actp, \
             tc.tile_pool(name="tmp", bufs=4) as tmpp, \
             tc.tile_pool(name="small", bufs=8) as smallp, \
             tc.tile_pool(name="cbfp", bufs=4) as cbfp, \
             tc.tile_pool(name="outp", bufs=3) as outp, \
             tc.tile_pool(name="ps_proj", bufs=3, space="PSUM") as psproj, \
             tc.tile_pool(name="ps_s", bufs=2, space="PSUM") as pss, \
             tc.tile_pool(name="ps_misc", bufs=3, space="PSUM") as psmisc:

            # ---- resident constants ----
            wq_sb = constp.tile([128, KT, F], BF16, tag="wq")
            wk_sb = constp.tile([128, KT, F], BF16, tag="wk")
            wv_sb = constp.tile([128, KT, F], BF16, tag="wv")
            wo_sb = constp.tile([128, PAIRS, DIM], BF16, tag="wo")
            nc.gpsimd.dma_start(out=wq_sb[:], in_=wq.rearrange("(kt p) f -> p kt f", p=128))
            nc.gpsimd.dma_start(out=wk_sb[:], in_=wk.rearrange("(kt p) f -> p kt f", p=128))
            nc.gpsimd.dma_start(out=wv_sb[:], in_=wv.rearrange("(kt p) f -> p kt f", p=128))
            nc.gpsimd.dma_start(out=wo_sb[:], in_=wo.rearrange("(ft p) n -> p ft n", p=128))

            ones_sum = constp.tile([128, 2], BF16, tag="ones_sum")
            nc.vector.memset(ones_sum[:], 0.0)
            nc.vector.memset(ones_sum[0:64, 0:1], 1.0)
            nc.vector.memset(ones_sum[64:128, 1:2], 1.0)
            ones_repl = constp.tile([2, 128], BF16, tag="ones_repl")
            nc.gpsimd.dma_start(out=ones_repl[:], in_=orp[:])

            # running context+ksum per head pair: [2*64 d, 64 e + 1 ksum]
            c_cur = constp.tile([128, PAIRS, E + 1], F32, tag="c_cur")
            nc.vector.memset(c_cur[:], 0.0)

            state = {}

            def emit_proj(c):
                xT = xtp.tile([128, KT, CH], BF16, tag="xT")
                for kt in range(KT):
                    nc.sync.dma_start_transpose(
                        out=xT[:, kt, :],
                        in_=x[c * CH:(c + 1) * CH, kt * 128:(kt + 1) * 128])

                E_sb = actp.tile([128, PAIRS, CH], BF16, tag="E")
                for p in range(PAIRS):
                    qt = psproj.tile([128, CH], F32, tag="proj")
                    for kt in range(KT):
                        nc.tensor.matmul(qt[:], wq_sb[:, kt, p * 128:(p + 1) * 128],
                                         xT[:, kt, :], start=(kt == 0), stop=(kt == KT - 1))
                    nc.scalar.activation(out=E_sb[:, p, :], in_=qt[:], func=AF.Exp)

                psik = actp.tile([128, PAIRS, CH], BF16, tag="psik")
                v_sb = actp.tile([128, PAIRS, HC * (E + 1) // PAIRS * PAIRS], BF16, tag="v")
                # v_sb free layout per tok-tile: 8 heads x 65 (64 v + ones col)
                for tt in range(PAIRS):  # 4 token tiles of 128
                    kp = psproj.tile([128, F], F32, tag="proj")
                    for kt in range(KT):
                        nc.tensor.matmul(kp[:], xT[:, kt, tt * 128:(tt + 1) * 128],
                                         wk_sb[:, kt, :], start=(kt == 0), stop=(kt == KT - 1))
                    tm = tmpp.tile([128, F], F32, tag="tm")
                    nc.vector.tensor_scalar_min(tm[:], kp[:], 0.0)
                    tm2 = tmpp.tile([128, F], F32, tag="tm2")
                    nc.scalar.activation(out=tm2[:], in_=tm[:], func=AF.Exp)
                    # psi = max(k,0) + exp(min(k,0))
                    nc.vector.scalar_tensor_tensor(
                        out=psik[:, tt, :], in0=kp[:], scalar=0.0, in1=tm2[:],
                        op0=OP.max, op1=OP.add)

                    vp = psproj.tile([128, F], F32, tag="proj")
                    for kt in range(KT):
                        nc.tensor.matmul(vp[:], xT[:, kt, tt * 128:(tt + 1) * 128],
                                         wv_sb[:, kt, :], start=(kt == 0), stop=(kt == KT - 1))
                    v3 = v_sb[:, tt, :].rearrange("p (h e1) -> p h e1", e1=E + 1)
                    nc.vector.tensor_copy(
                        out=v3[:, :, 0:E],
                        in_=vp[:].rearrange("p (h e) -> p h e", e=E))
                    nc.vector.memset(v3[:, :, E:E + 1], 1.0)
                state[c] = (xT, E_sb, psik, v_sb)

            def emit_attn(c):
                xT, E_sb, psik, v_sb = state.pop(c)
                atn = actp.tile([128, PAIRS, CH], BF16, tag="atn")
                for p in range(PAIRS):
                    # softmax denominator: per-head column sums of exp(q)
                    sm = psmisc.tile([128, CH], F32, tag="misc")
                    nc.tensor.matmul(sm[0:2, :], ones_sum[:], E_sb[:, p, :],
                                     start=True, stop=True)
                    rf = smallp.tile([2, CH], F32, tag="rf")
                    nc.vector.reciprocal(out=rf[:], in_=sm[0:2, :])
                    rbf = smallp.tile([2, CH], BF16, tag="rbf")
                    nc.vector.tensor_copy(out=rbf[:], in_=rf[:])
                    repl = psmisc.tile([128, CH], F32, tag="misc")
                    nc.tensor.matmul(repl[:], ones_repl[:], rbf[:], start=True, stop=True)

                    # context matmuls: S_j = psi_bu^T @ [v_bu | 1]
                    s_ev = pss.tile([128, UC // 2, E + 1], F32, tag="s")
                    s_od = pss.tile([128, UC // 2, E + 1], F32, tag="s")
                    for j in range(UC):
                        sdst = s_ev if j % 2 == 0 else s_od
                        row = (j % 2) * 64
                        tt, r0 = j // 2, (j % 2) * 64
                        for hh in range(2):
                            h = 2 * p + hh
                            nc.tensor.matmul(
                                sdst[hh * 64:(hh + 1) * 64, j // 2, :],
                                psik[r0:r0 + 64, tt, h * E:(h + 1) * E],
                                v_sb[r0:r0 + 64, tt, :].rearrange(
                                    "p (g e1) -> p g e1", e1=E + 1)[:, h, :],
                                start=True, stop=True,
                                tile_position=(row, hh * 64))

                    # ksum prefix along buckets (exclusive via shift)
                    ks = smallp.tile([128, UC], F32, tag="ks")
                    k2 = ks[:].rearrange("p (a b) -> p a b", b=2)
                    nc.vector.tensor_copy(out=k2[:, :, 0:1], in_=s_ev[:, :, E:E + 1])
                    nc.vector.tensor_copy(out=k2[:, :, 1:2], in_=s_od[:, :, E:E + 1])
                    ksc = smallp.tile([128, UC], F32, tag="ksc")
                    nc.vector.tensor_tensor_scan(
                        out=ksc[:], data0=ks[:], data1=ks[:],
                        initial=c_cur[:, p, E:E + 1], op0=OP.add, op1=OP.bypass)
                    rs = smallp.tile([128, UC], F32, tag="rs")
                    nc.vector.tensor_copy(out=rs[:, 1:UC], in_=ksc[:, 0:UC - 1])
                    nc.vector.tensor_copy(out=rs[:, 0:1], in_=c_cur[:, p, E:E + 1])
                    R = smallp.tile([128, UC], F32, tag="R")
                    nc.vector.tensor_scalar_add(rs[:], rs[:], EPS)
                    nc.vector.reciprocal(out=R[:], in_=rs[:])
                    if c == 0:
                        nc.vector.memset(R[:, 0:1], 0.0)  # bucket-0 blindspot

                    # q'' = exp(q) * softmax_recip * ksum_recip
                    tq = tmpp.tile([128, CH], F32, tag="tq")
                    nc.vector.tensor_tensor(out=tq[:], in0=E_sb[:, p, :], in1=repl[:],
                                            op=OP.mult)
                    q2 = tmpp.tile([128, CH], BF16, tag="q2")
                    rap = R[:]
                    import concourse.bass as bass_mod
                    Rb = bass_mod.AP(tensor=rap.tensor, offset=rap.offset,
                                     ap=[rap.ap[0], rap.ap[1], [0, BUCKET]])
                    nc.vector.tensor_tensor(
                        out=q2[:].rearrange("p (u t) -> p u t", t=BUCKET),
                        in0=tq[:].rearrange("p (u t) -> p u t", t=BUCKET),
                        in1=Rb, op=OP.mult)

                    # walk buckets: snapshot C, attn matmul, then C += S_j
                    at = psmisc.tile([128, CH], F32, tag="misc")
                    for j in range(UC):
                        cbf = cbfp.tile([128, E + 1], BF16, tag="cbf")
                        nc.vector.tensor_copy(out=cbf[:], in_=c_cur[:, p, :])
                        for hh in range(2):
                            nc.tensor.matmul(
                                at[hh * 64:(hh + 1) * 64, j * 64:(j + 1) * 64],
                                cbf[hh * 64:(hh + 1) * 64, 0:E],
                                q2[hh * 64:(hh + 1) * 64, j * 64:(j + 1) * 64],
                                start=True, stop=True,
                                tile_position=(hh * 64, hh * 64))
                        sj = s_ev if j % 2 == 0 else s_od
                        nc.vector.tensor_add(c_cur[:, p, :], c_cur[:, p, :],
                                             sj[:, j // 2, :])
                    nc.vector.tensor_copy(out=atn[:, p, :], in_=at[:])

                # output projection: out_chunk = attn.T^T @ Wo (contract feats)
                for tt in range(PAIRS):
                    osb = outp.tile([128, DIM], F32, tag="osb")
                    for half in range(2):
                        op_ = psproj.tile([128, 512], F32, tag="proj")
                        for p in range(PAIRS):
                            nc.tensor.matmul(
                                op_[:], atn[:, p, tt * 128:(tt + 1) * 128],
                                wo_sb[:, p, half * 512:(half + 1) * 512],
                                start=(p == 0), stop=(p == PAIRS - 1))
                        nc.vector.tensor_copy(out=osb[:, half * 512:(half + 1) * 512],
                                              in_=op_[:])
                    nc.gpsimd.dma_start(
                        out=out[c * CH + tt * 128:c * CH + (tt + 1) * 128, :],
                        in_=osb[:])

            for c in range(n_chunks + 1):
                if c < n_chunks:
                    emit_proj(c)
                if c >= 1:
                    emit_attn(c - 1)

    nc.finalize()
    return nc


def _orp():
    m = np.zeros((2, 128), dtype=ml_dtypes.bfloat16)
    m[0, 0:64] = 1
    m[1, 64:128] = 1
    return m


def kernel(x, Wq, Wk, Wv, Wo, bo):
    from concourse.bass_utils import run_bass_kernel_spmd

    if "nc" not in _NC_CACHE:
        _NC_CACHE["nc"] = build_nc()
    nc = _NC_CACHE["nc"]

    bf = ml_dtypes.bfloat16
    x = np.asarray(x)
    Wq, Wk, Wv, Wo = (np.asarray(w) for w in (Wq, Wk, Wv, Wo))
    in_maps = []
    for c in range(8):
        b, g = c // 2, c % 2
        sl = slice(g * F, (g + 1) * F)
        in_maps.append({
            "x": x[b].astype(bf),
            "wq": Wq[:, sl].astype(bf),
            "wk": Wk[:, sl].astype(bf),
            "wv": Wv[:, sl].astype(bf),
            "wo": Wo[sl, :].astype(bf),
            "orp": _orp(),
        })
    res = run_bass_kernel_spmd(nc, in_maps, core_ids=list(range(8)))
    outs = [res.results[c]["out"] for c in range(8)]
    full = np.stack([outs[2 * b] + outs[2 * b + 1] for b in range(B)], axis=0)
    return (full + np.asarray(bo)[None, None, :]).astype(np.float32)



# revision 2
# speedup vs baseline: 1.2854x; 1.2854x over previous
"""Trainium2 Bass kernel for bucketed causal linear self-attention.

Model (B=4, T=4096, DIM=1024, H=16 heads, E=64, BUCKET=64):
  q,k,v = x@Wq, x@Wk, x@Wv ; q softmaxed over head-dim, k -> elu(k)+1
  per-bucket context C_u = cumsum_u(k_bu^T v_bu), normalized by cumsum of
  key-sums, shifted one bucket; attn_bu = q_bu @ C_{u-1}; out = attn@Wo + bo.

Sharding over 8 cores: core c -> batch c//2, head-group c%2 (8 heads = 512
feats). q/k/v column-sharded by head, Wo row-sharded; host sums the two
partial outputs per batch (all-reduce on host) and adds bo.

Per-core structure (v2 — bucket cumsum moved onto the tensor engine):
  attn^T[e, tok] accumulates in PSUM as
      C_carry^T @ q2          (all 512 chunk tokens)
    + sum_j S_j^T @ q2[:, tokens in buckets > j]   (shrinking-N matmuls)
  so no per-bucket DVE snapshot/add chain exists.  S_j = psi_bu^T [v|1]
  per bucket; a separate 8-matmul group computes the chunk context total
  C_inc for the running carry.  Softmax reciprocal = exp(-ln(sum)) on the
  scalar engine; ksum reciprocal (per bucket) stays on DVE (FD=8).
  q'' = exp(q) * softmax_recip * 1/(ksum_prefix+eps), with the bucket-0
  blindspot via a zeroed scale column on chunk 0.
"""

import sys
import numpy as np
import ml_dtypes

sys.path.insert(0, "/opt/trn_rl_repo")

B, T, DIM, H, BUCKET = 4, 4096, 1024, 16, 64
E = 64           # head dim
HC = 8           # heads per core
F = HC * E       # per-core feature width = 512
CH = 512         # tokens per chunk
UC = CH // BUCKET  # buckets per chunk = 8
PAIRS = HC // 2  # head pairs = 4
KT = DIM // 128  # contraction tiles = 8
EPS = 1e-6

_NC_CACHE = {}


def build_nc(n_chunks=T // CH):
    import concourse.bass as bass
    import concourse.mybir as mybir
    from concourse import bacc
    from concourse.tile import TileContext

    BF16 = mybir.dt.bfloat16
    F32 = mybir.dt.float32
    AF = mybir.ActivationFunctionType
    OP = mybir.AluOpType

    Tt = n_chunks * CH

    nc = bacc.Bacc("TRN2", target_bir_lowering=False, debug=False, num_devices=8)
    x = nc.dram_tensor("x", [Tt, DIM], BF16, kind="ExternalInput").ap()
    wq = nc.dram_tensor("wq", [DIM, F], BF16, kind="ExternalInput").ap()
    wk = nc.dram_tensor("wk", [DIM, F], BF16, kind="ExternalInput").ap()
    wv = nc.dram_tensor("wv", [DIM, F], BF16, kind="ExternalInput").ap()
    wo = nc.dram_tensor("wo", [F, DIM], BF16, kind="ExternalInput").ap()
    orp = nc.dram_tensor("orp", [128, 128], BF16, kind="ExternalInput").ap()
    out = nc.dram_tensor("out", [Tt, DIM], BF16, kind="ExternalOutput").ap()

    with TileContext(nc) as tc:
        with tc.tile_pool(name="const", bufs=1) as constp, \
             tc.tile_pool(name="xt", bufs=2) as xtp, \
             tc.tile_pool(name="act", bufs=2) as actp, \
             tc.tile_pool(name="tmp", bufs=3) as tmpp, \
             tc.tile_pool(name="small", bufs=8) as smallp, \
             tc.tile_pool(name="outp", bufs=2) as outp, \
             tc.tile_pool(name="ps_proj", bufs=2, space="PSUM") as psproj, \
             tc.tile_pool(name="ps_s", bufs=1, space="PSUM") as pss, \
             tc.tile_pool(name="ps_c", bufs=1, space="PSUM") as psc, \
             tc.tile_pool(name="ps_attn", bufs=1, space="PSUM") as psattn, \
             tc.tile_pool(name="ps_sm", bufs=1, space="PSUM") as pssm, \
             tc.tile_pool(name="ps_repl", bufs=1, space="PSUM") as psrepl:

            # ---- resident constants ----
            wq_sb = constp.tile([128, KT, F], BF16, tag="wq")
            wk_sb = constp.tile([128, KT, F], BF16, tag="wk")
            wv_sb = constp.tile([128, KT, F], BF16, tag="wv")
            wo_sb = constp.tile([128, PAIRS, DIM], BF16, tag="wo")
            nc.gpsimd.dma_start(out=wq_sb[:], in_=wq.rearrange("(kt p) f -> p kt f", p=128))
            nc.gpsimd.dma_start(out=wk_sb[:], in_=wk.rearrange("(kt p) f -> p kt f", p=128))
            nc.gpsimd.dma_start(out=wv_sb[:], in_=wv.rearrange("(kt p) f -> p kt f", p=128))
            nc.gpsimd.dma_start(out=wo_sb[:], in_=wo.rearrange("(ft p) n -> p ft n", p=128))

            ones_sum = constp.tile([128, 2], BF16, tag="ones_sum")
            nc.vector.memset(ones_sum[:], 0.0)
            nc.vector.memset(ones_sum[0:64, 0:1], 1.0)
            nc.vector.memset(ones_sum[64:128, 1:2], 1.0)
            # orp[32p+0, 0:64]=1, orp[32p+1, 64:128]=1 (host-built)
            orp_sb = constp.tile([128, 128], BF16, tag="orp")
            nc.gpsimd.dma_start(out=orp_sb[:], in_=orp[:])

            # running context (+ ksum col 64) per pair, f32 master + bf16 copy
            c_ms = constp.tile([128, PAIRS, E + 1], F32, tag="c_ms")
            nc.vector.memset(c_ms[:], 0.0)
            c_bf = constp.tile([128, PAIRS, E], BF16, tag="c_bf")
            nc.vector.memset(c_bf[:], 0.0)

            state = {}

            def emit_proj(c):
                xT = xtp.tile([128, KT, CH], BF16, tag="xT")
                for kt in range(KT):
                    nc.sync.dma_start_transpose(
                        out=xT[:, kt, :],
                        in_=x[c * CH:(c + 1) * CH, kt * 128:(kt + 1) * 128])

                # q^T, exp(q), and per-token softmax sums (all pairs in one PSUM)
                E_sb = actp.tile([128, PAIRS, CH], BF16, tag="E")
                sm = pssm.tile([128, CH], F32, tag="sm")
                for p in range(PAIRS):
                    qt = psproj.tile([128, CH], F32, tag="proj")
                    for kt in range(KT):
                        nc.tensor.matmul(qt[:], wq_sb[:, kt, p * 128:(p + 1) * 128],
                                         xT[:, kt, :], start=(kt == 0), stop=(kt == KT - 1))
                    nc.scalar.activation(out=E_sb[:, p, :], in_=qt[:], func=AF.Exp)
                    nc.tensor.matmul(sm[32 * p:32 * p + 2, :], ones_sum[:], E_sb[:, p, :],
                                     start=True, stop=True, tile_position=(0, 32 * p))
                # softmax reciprocal = exp(-ln(sum)) on the scalar engine
                lnt = tmpp.tile([128, CH], F32, tag="lnt")
                nc.scalar.activation(out=lnt[:], in_=sm[:], func=AF.Ln)
                recip_sb = actp.tile([128, CH], BF16, tag="recip")
                nc.scalar.activation(out=recip_sb[:], in_=lnt[:], func=AF.Exp, scale=-1.0)

                psik = actp.tile([128, PAIRS, CH], BF16, tag="psik")
                v_sb = actp.tile([128, PAIRS, HC * (E + 1) // PAIRS * PAIRS], BF16, tag="v")
                # v_sb free layout per tok-tile: 8 heads x 65 (64 v + ones col)
                for tt in range(PAIRS):  # 4 token tiles of 128
                    kp = psproj.tile([128, F], F32, tag="proj")
                    for kt in range(KT):
                        nc.tensor.matmul(kp[:], xT[:, kt, tt * 128:(tt + 1) * 128],
                                         wk_sb[:, kt, :], start=(kt == 0), stop=(kt == KT - 1))
                    tm = tmpp.tile([128, F], F32, tag="tm")
                    nc.vector.tensor_scalar_min(tm[:], kp[:], 0.0)
                    tm2 = tmpp.tile([128, F], F32, tag="tm2")
                    nc.scalar.activation(out=tm2[:], in_=tm[:], func=AF.Exp)
                    # psi = max(k,0) + exp(min(k,0))
                    nc.vector.scalar_tensor_tensor(
                        out=psik[:, tt, :], in0=kp[:], scalar=0.0, in1=tm2[:],
                        op0=OP.max, op1=OP.add)

                    vp = psproj.tile([128, F], F32, tag="proj")
                    for kt in range(KT):
                        nc.tensor.matmul(vp[:], xT[:, kt, tt * 128:(tt + 1) * 128],
                                         wv_sb[:, kt, :], start=(kt == 0), stop=(kt == KT - 1))
                    v3 = v_sb[:, tt, :].rearrange("p (h e1) -> p h e1", e1=E + 1)
                    nc.scalar.activation(
                        out=v3[:, :, 0:E],
                        in_=vp[:].rearrange("p (h e) -> p h e", e=E), func=AF.Copy)
                    nc.vector.memset(v3[:, :, E:E + 1], 1.0)
                state[c] = (E_sb, recip_sb, psik, v_sb)

            def emit_attn(c):
                E_sb, recip_sb, psik, v_sb = state.pop(c)
                atn = actp.tile([128, PAIRS, CH], BF16, tag="atn")
                for p in range(PAIRS):
                    # per-bucket context matmuls: S_j = psi_bu^T @ [v_bu | 1]
                    s_ev = pss.tile([128, UC // 2, E + 1], F32, tag="s_ev")
                    s_od = pss.tile([128, UC // 2, E + 1], F32, tag="s_od")
                    for j in range(UC):
                        sdst = s_ev if j % 2 == 0 else s_od
                        tt, r0 = j // 2, (j % 2) * 64
                        for hh in range(2):
                            h = 2 * p + hh
                            nc.tensor.matmul(
                                sdst[hh * 64:(hh + 1) * 64, j // 2, :],
                                psik[r0:r0 + 64, tt, h * E:(h + 1) * E],
                                v_sb[r0:r0 + 64, tt, :].rearrange(
                                    "p (g e1) -> p g e1", e1=E + 1)[:, h, :],
                                start=True, stop=True,
                                tile_position=(r0, hh * 64))

                    # chunk context total (for the running carry), incl ksum col
                    c_inc = psc.tile([128, E + 1], F32, tag="c_inc")
                    for tt in range(PAIRS):
                        for hh in range(2):
                            h = 2 * p + hh
                            nc.tensor.matmul(
                                c_inc[hh * 64:(hh + 1) * 64, :],
                                psik[:, tt, h * E:(h + 1) * E],
                                v_sb[:, tt, :].rearrange(
                                    "p (g e1) -> p g e1", e1=E + 1)[:, h, :],
                                start=(tt == 0), stop=(tt == PAIRS - 1),
                                tile_position=(0, hh * 64))

                    # ksum exclusive prefix along buckets -> per-bucket scale R
                    ks = smallp.tile([128, UC], F32, tag="ks")
                    k2 = ks[:].rearrange("p (a b) -> p a b", b=2)
                    nc.vector.tensor_copy(out=k2[:, :, 0:1], in_=s_ev[:, :, E:E + 1])
                    nc.vector.tensor_copy(out=k2[:, :, 1:2], in_=s_od[:, :, E:E + 1])
                    ksc = smallp.tile([128, UC], F32, tag="ksc")
                    nc.vector.tensor_tensor_scan(
                        out=ksc[:], data0=ks[:], data1=ks[:],
                        initial=c_ms[:, p, E:E + 1], op0=OP.add, op1=OP.bypass)
                    rs = smallp.tile([128, UC], F32, tag="rs")
                    nc.vector.tensor_copy(out=rs[:, 1:UC], in_=ksc[:, 0:UC - 1])
                    nc.vector.tensor_copy(out=rs[:, 0:1], in_=c_ms[:, p, E:E + 1])
                    R = smallp.tile([128, UC], F32, tag="R")
                    nc.vector.tensor_scalar_add(rs[:], rs[:], EPS)
                    nc.vector.reciprocal(out=R[:], in_=rs[:])
                    if c == 0:
                        nc.vector.memset(R[:, 0:1], 0.0)  # bucket-0 blindspot

                    # q'' = exp(q) * softmax_recip * ksum_recip
                    repl = psrepl.tile([128, CH], F32, tag="repl")
                    nc.tensor.matmul(repl[:], orp_sb[32 * p:32 * p + 2, :],
                                     recip_sb[32 * p:32 * p + 2, :],
                                     start=True, stop=True, tile_position=(32 * p, 0))
                    import concourse.bass as bass_mod
                    rap = R[:]
                    Rb = bass_mod.AP(tensor=rap.tensor, offset=rap.offset,
                                     ap=[rap.ap[0], rap.ap[1], [0, BUCKET]])
                    RR = tmpp.tile([128, CH], BF16, tag="RR")
                    nc.vector.tensor_tensor(
                        out=RR[:].rearrange("p (u t) -> p u t", t=BUCKET),
                        in0=repl[:].rearrange("p (u t) -> p u t", t=BUCKET),
                        in1=Rb, op=OP.mult)
                    q2 = tmpp.tile([128, CH], BF16, tag="q2")
                    nc.vector.tensor_tensor(out=q2[:], in0=E_sb[:, p, :], in1=RR[:],
                                            op=OP.mult)

                    # S_j -> bf16 (buckets 0..6 only; bucket 7 never feeds attn)
                    s_bf = tmpp.tile([128, PAIRS, 2, E], BF16, tag="s_bf")
                    nc.vector.tensor_copy(out=s_bf[:, :, 0, :], in_=s_ev[:, :, 0:E])
                    nc.vector.tensor_copy(out=s_bf[:, 0:3, 1, :], in_=s_od[:, 0:3, 0:E])

                    # attn^T accumulation: carry over all tokens, then each
                    # S_j over tokens in buckets > j
                    at = psattn.tile([128, CH], F32, tag="attn")
                    for hh in range(2):
                        r0 = hh * 64
                        nc.tensor.matmul(
                            at[r0:r0 + 64, :], c_bf[r0:r0 + 64, p, :],
                            q2[r0:r0 + 64, :], start=True, stop=False,
                            tile_position=(r0, r0))
                        for j in range(UC - 1):
                            q0 = (j + 1) * BUCKET
                            nc.tensor.matmul(
                                at[r0:r0 + 64, q0:CH],
                                s_bf[r0:r0 + 64, j // 2, j % 2, :],
                                q2[r0:r0 + 64, q0:CH], start=False,
                                stop=(j == UC - 2), tile_position=(r0, r0))
                    nc.scalar.activation(out=atn[:, p, :], in_=at[:], func=AF.Copy)

                    # advance the running carry (after sc an/carry reads above)
                    nc.vector.tensor_tensor(out=c_ms[:, p, :], in0=c_ms[:, p, :],
                                            in1=c_inc[:], op=OP.add)
                    nc.vector.tensor_copy(out=c_bf[:, p, :], in_=c_ms[:, p, 0:E])

                # output projection: out_chunk = attn^T.T @ Wo (contract feats)
                osb = outp.tile([128, PAIRS, DIM], BF16, tag="osb")
                for tt in range(PAIRS):
                    for half in range(2):
                        op_ = psproj.tile([128, 512], F32, tag="proj")
                        for p in range(PAIRS):
                            nc.tensor.matmul(
                                op_[:], atn[:, p, tt * 128:(tt + 1) * 128],
                                wo_sb[:, p, half * 512:(half + 1) * 512],
                                start=(p == 0), stop=(p == PAIRS - 1))
                        nc.scalar.activation(
                            out=osb[:, tt, half * 512:(half + 1) * 512],
                            in_=op_[:], func=AF.Copy)
                nc.gpsimd.dma_start(
                    out=out[c * CH:(c + 1) * CH, :].rearrange(
                        "(tt p) d -> p tt d", p=128),
                    in_=osb[:])

            for c in range(n_chunks + 1):
                if c < n_chunks:
                    emit_proj(c)
                if c >= 1:
                    emit_attn(c - 1)

    nc.finalize()
    return nc


def _orp():
    m = np.zeros((128, 128), dtype=ml_dtypes.bfloat16)
    for p in range(PAIRS):
        m[32 * p, 0:64] = 1
        m[32 * p + 1, 64:128] = 1
    return m


def build_in_maps(x, Wq, Wk, Wv, Wo):
    bf = ml_dtypes.bfloat16
    x = np.asarray(x)
    Wq, Wk, Wv, Wo = (np.asarray(w) for w in (Wq, Wk, Wv, Wo))
    in_maps = []
    for c in range(8):
        b, g = c // 2, c % 2
        sl = slice(g * F, (g + 1) * F)
        in_maps.append({
            "x": x[b].astype(bf),
            "wq": Wq[:, sl].astype(bf),
            "wk": Wk[:, sl].astype(bf),
            "wv": Wv[:, sl].astype(bf),
            "wo": Wo[sl, :].astype(bf),
            "orp": _orp(),
        })
    return in_maps


def kernel(x, Wq, Wk, Wv, Wo, bo):
    from concourse.bass_utils import run_bass_kernel_spmd

    if "nc" not in _NC_CACHE:
        _NC_CACHE["nc"] = build_nc()
    nc = _NC_CACHE["nc"]

    in_maps = build_in_maps(x, Wq, Wk, Wv, Wo)
    res = run_bass_kernel_spmd(nc, in_maps, core_ids=list(range(8)))
    outs = [res.results[c]["out"].astype(np.float32) for c in range(8)]
    full = np.stack([outs[2 * b] + outs[2 * b + 1] for b in range(B)], axis=0)
    return (full + np.asarray(bo)[None, None, :].astype(np.float32)).astype(np.float32)


# revision 5
# speedup vs baseline: 1.3950x; 1.0853x over previous
"""Trainium2 Bass kernel for bucketed causal linear self-attention.

Model (B=4, T=4096, DIM=1024, H=16 heads, E=64, BUCKET=64):
  q,k,v = x@Wq, x@Wk, x@Wv ; q softmaxed over head-dim, k -> elu(k)+1
  per-bucket context C_u = cumsum_u(k_bu^T v_bu), normalized by cumsum of
  key-sums, shifted one bucket; attn_bu = q_bu @ C_{u-1}; out = attn@Wo + bo.

Sharding over 8 cores: core c -> batch c//2, head-group c%2 (8 heads = 512
feats). q/k/v column-sharded by head, Wo row-sharded; host sums the two
partial outputs per batch (all-reduce on host) and adds bo.

v3 structure per core:
  x arrives HOST-TRANSPOSED [DIM, T] so x^T tiles load as plain DMAs (the
  v2 dma_start_transpose chain serialized on the xbar and re-throttled the
  PE clock).  attn^T accumulates on the tensor engine:
      C_carry^T @ q2  +  sum_j S_j^T @ q2[:, buckets > j]
  (shrinking-N matmuls; no per-bucket DVE walk).  The chunk context
  increment is a DVE tensor_reduce over the bf16 S tile; the bucket-ksum
  prefix scan runs on the same tile.  Softmax reciprocal is one DVE
  reciprocal per chunk; its per-token broadcast is a 2-row matmul.
  q'' = exp(q) * softmax_recip * 1/(ksum_prefix+eps), bucket-0 blindspot
  via a zeroed scale column on chunk 0.  Output is bf16; host sums the
  two per-batch partials in f32 and adds bo.
"""

import sys
import numpy as np
import ml_dtypes

sys.path.insert(0, "/opt/trn_rl_repo")

B, T, DIM, H, BUCKET = 4, 4096, 1024, 16, 64
E = 64           # head dim
HC = 8           # heads per core
F = HC * E       # per-core feature width = 512
CH = 512         # tokens per chunk
UC = CH // BUCKET  # buckets per chunk = 8
PAIRS = HC // 2  # head pairs = 4
KT = DIM // 128  # contraction tiles = 8
EPS = 1e-6

_NC_CACHE = {}


def build_nc(n_chunks=T // CH):
    import concourse.bass as bass
    import concourse.mybir as mybir
    from concourse import bacc
    from concourse.tile import TileContext

    BF16 = mybir.dt.bfloat16
    F32 = mybir.dt.float32
    AF = mybir.ActivationFunctionType
    OP = mybir.AluOpType

    Tt = n_chunks * CH

    nc = bacc.Bacc("TRN2", target_bir_lowering=False, debug=False, num_devices=8)
    x = nc.dram_tensor("x", [DIM, Tt], BF16, kind="ExternalInput").ap()
    wq = nc.dram_tensor("wq", [DIM, F], BF16, kind="ExternalInput").ap()
    wk = nc.dram_tensor("wk", [DIM, F], BF16, kind="ExternalInput").ap()
    wv = nc.dram_tensor("wv", [DIM, F], BF16, kind="ExternalInput").ap()
    wo = nc.dram_tensor("wo", [F, DIM], BF16, kind="ExternalInput").ap()
    orp = nc.dram_tensor("orp", [128, 128], BF16, kind="ExternalInput").ap()
    out = nc.dram_tensor("out", [Tt, DIM], BF16, kind="ExternalOutput").ap()

    with TileContext(nc) as tc:
        with tc.tile_pool(name="const", bufs=1) as constp, \
             tc.tile_pool(name="xt", bufs=2) as xtp, \
             tc.tile_pool(name="act", bufs=2) as actp, \
             tc.tile_pool(name="tmp", bufs=3) as tmpp, \
             tc.tile_pool(name="small", bufs=8) as smallp, \
             tc.tile_pool(name="outp", bufs=2) as outp, \
             tc.tile_pool(name="ps_proj", bufs=2, space="PSUM") as psproj, \
             tc.tile_pool(name="ps_s", bufs=1, space="PSUM") as pss, \
             tc.tile_pool(name="ps_attn", bufs=2, space="PSUM") as psattn, \
             tc.tile_pool(name="ps_misc", bufs=2, space="PSUM") as psmisc:

            # ---- resident constants ----
            wq_sb = constp.tile([128, KT, F], BF16, tag="wq")
            wk_sb = constp.tile([128, KT, F], BF16, tag="wk")
            wv_sb = constp.tile([128, KT, F], BF16, tag="wv")
            wo_sb = constp.tile([128, PAIRS, DIM], BF16, tag="wo")
            nc.gpsimd.dma_start(out=wq_sb[:], in_=wq.rearrange("(kt p) f -> p kt f", p=128))
            nc.gpsimd.dma_start(out=wk_sb[:], in_=wk.rearrange("(kt p) f -> p kt f", p=128))
            nc.gpsimd.dma_start(out=wv_sb[:], in_=wv.rearrange("(kt p) f -> p kt f", p=128))
            nc.gpsimd.dma_start(out=wo_sb[:], in_=wo.rearrange("(ft p) n -> p ft n", p=128))

            ones_sum = constp.tile([128, 2], BF16, tag="ones_sum")
            nc.vector.memset(ones_sum[:], 0.0)
            nc.vector.memset(ones_sum[0:64, 0:1], 1.0)
            nc.vector.memset(ones_sum[64:128, 1:2], 1.0)
            # orp[32p+0, 0:64]=1, orp[32p+1, 64:128]=1 (host-built)
            orp_sb = constp.tile([128, 128], BF16, tag="orp")
            nc.gpsimd.dma_start(out=orp_sb[:], in_=orp[:])

            # running context (+ ksum col 64) per pair, f32 master + bf16 copy
            c_ms = constp.tile([128, PAIRS, E + 1], F32, tag="c_ms")
            nc.vector.memset(c_ms[:], 0.0)
            c_bf = constp.tile([128, PAIRS, E], BF16, tag="c_bf")
            nc.vector.memset(c_bf[:], 0.0)

            state = {}

            def emit_proj(c):
                xT = xtp.tile([128, KT, CH], BF16, tag="xT")
                nc.gpsimd.dma_start(
                    out=xT[:],
                    in_=x[:, c * CH:(c + 1) * CH].rearrange(
                        "(kt p) t -> p kt t", p=128))

                # q^T, exp(q), per-token softmax sums (rows 32p..32p+2 of sm)
                E_sb = actp.tile([128, PAIRS, CH], BF16, tag="E")
                sm = psmisc.tile([128, CH], F32, tag="misc")
                for p in range(PAIRS):
                    qt = psproj.tile([128, CH], F32, tag="proj")
                    for kt in range(KT):
                        nc.tensor.matmul(qt[:], wq_sb[:, kt, p * 128:(p + 1) * 128],
                                         xT[:, kt, :], start=(kt == 0), stop=(kt == KT - 1))
                    nc.scalar.activation(out=E_sb[:, p, :], in_=qt[:], func=AF.Exp)
                    nc.tensor.matmul(sm[32 * p:32 * p + 2, :], ones_sum[:], E_sb[:, p, :],
                                     start=True, stop=True, tile_position=(0, 32 * p))
                recip_sb = actp.tile([128, CH], BF16, tag="recip")
                with nc.allow_low_precision(reason="bf16 softmax recip, 4e-3 rel"):
                    nc.vector.reciprocal(out=recip_sb[0:98, :], in_=sm[0:98, :])

                psik = actp.tile([128, PAIRS, CH], BF16, tag="psik")
                v_sb = actp.tile([128, PAIRS, HC * (E + 1) // PAIRS * PAIRS], BF16, tag="v")
                # v_sb free layout per tok-tile: 8 heads x 65 (64 v + ones col)
                for tt in range(PAIRS):  # 4 token tiles of 128
                    kp = psproj.tile([128, F], F32, tag="proj")
                    for kt in range(KT):
                        nc.tensor.matmul(kp[:], xT[:, kt, tt * 128:(tt + 1) * 128],
                                         wk_sb[:, kt, :], start=(kt == 0), stop=(kt == KT - 1))
                    tm = tmpp.tile([128, F], F32, tag="tm")
                    nc.vector.tensor_scalar_min(tm[:], kp[:], 0.0)
                    tm2 = tmpp.tile([128, F], F32, tag="tm2")
                    nc.scalar.activation(out=tm2[:], in_=tm[:], func=AF.Exp)
                    # psi = max(k,0) + exp(min(k,0))
                    nc.vector.scalar_tensor_tensor(
                        out=psik[:, tt, :], in0=kp[:], scalar=0.0, in1=tm2[:],
                        op0=OP.max, op1=OP.add)

                    vp = psproj.tile([128, F], F32, tag="proj")
                    for kt in range(KT):
                        nc.tensor.matmul(vp[:], xT[:, kt, tt * 128:(tt + 1) * 128],
                                         wv_sb[:, kt, :], start=(kt == 0), stop=(kt == KT - 1))
                    v3 = v_sb[:, tt, :].rearrange("p (h e1) -> p h e1", e1=E + 1)
                    nc.scalar.activation(
                        out=v3[:, :, 0:E],
                        in_=vp[:].rearrange("p (h e) -> p h e", e=E), func=AF.Copy)
                    nc.vector.memset(v3[:, :, E:E + 1], 1.0)
                state[c] = (E_sb, recip_sb, psik, v_sb)

            def emit_attn(c):
                E_sb, recip_sb, psik, v_sb = state.pop(c)
                atn = actp.tile([128, PAIRS, CH], BF16, tag="atn")
                for p in range(PAIRS):
                    # per-bucket context matmuls: S_j = psi_bu^T @ [v_bu | 1]
                    s_ev = pss.tile([128, UC // 2, E + 1], F32, tag="s_ev")
                    s_od = pss.tile([128, UC // 2, E + 1], F32, tag="s_od")
                    for j in range(UC):
                        sdst = s_ev if j % 2 == 0 else s_od
                        tt, r0 = j // 2, (j % 2) * 64
                        for hh in range(2):
                            h = 2 * p + hh
                            nc.tensor.matmul(
                                sdst[hh * 64:(hh + 1) * 64, j // 2, :],
                                psik[r0:r0 + 64, tt, h * E:(h + 1) * E],
                                v_sb[r0:r0 + 64, tt, :].rearrange(
                                    "p (g e1) -> p g e1", e1=E + 1)[:, h, :],
                                start=True, stop=True,
                                tile_position=(r0, hh * 64))

                    # all 8 buckets -> bf16 [128, (a 4) (b 2) 65]; b = j parity
                    s_bf = tmpp.tile([128, UC // 2, 2, E + 1], BF16, tag="s_bf")
                    nc.vector.tensor_copy(out=s_bf[:, :, 0, :], in_=s_ev[:])
                    nc.vector.tensor_copy(out=s_bf[:, :, 1, :], in_=s_od[:])

                    # ksum exclusive prefix along buckets -> per-bucket scale R
                    ksv = s_bf[:].rearrange("p a b e1 -> p (a b) e1")[:, :, E:E + 1] \
                        .rearrange("p j one -> p (j one)")
                    ksc = smallp.tile([128, UC], F32, tag="ksc")
                    nc.vector.tensor_tensor_scan(
                        out=ksc[:], data0=ksv, data1=ksv,
                        initial=c_ms[:, p, E:E + 1], op0=OP.add, op1=OP.bypass)
                    rs = smallp.tile([128, UC], F32, tag="rs")
                    nc.vector.tensor_copy(out=rs[:, 1:UC], in_=ksc[:, 0:UC - 1])
                    nc.vector.tensor_copy(out=rs[:, 0:1], in_=c_ms[:, p, E:E + 1])
                    R = smallp.tile([128, UC], BF16, tag="R")
                    nc.vector.tensor_scalar_add(rs[:], rs[:], EPS)
                    with nc.allow_low_precision(reason="bf16 ksum recip, 4e-3 rel"):
                        nc.vector.reciprocal(out=R[:], in_=rs[:])
                    if c == 0:
                        nc.vector.memset(R[:, 0:1], 0.0)  # bucket-0 blindspot

                    # q'' = exp(q) * softmax_recip * ksum_recip
                    repl = psmisc.tile([128, CH], F32, tag="misc")
                    nc.tensor.matmul(repl[:], orp_sb[32 * p:32 * p + 2, :],
                                     recip_sb[32 * p:32 * p + 2, :],
                                     start=True, stop=True, tile_position=(32 * p, 0))
                    import concourse.bass as bass_mod
                    rap = R[:]
                    Rb = bass_mod.AP(tensor=rap.tensor, offset=rap.offset,
                                     ap=[rap.ap[0], rap.ap[1], [0, BUCKET]])
                    RR = tmpp.tile([128, CH], BF16, tag="RR")
                    nc.vector.tensor_tensor(
                        out=RR[:].rearrange("p (u t) -> p u t", t=BUCKET),
                        in0=repl[:].rearrange("p (u t) -> p u t", t=BUCKET),
                        in1=Rb, op=OP.mult)
                    q2 = tmpp.tile([128, CH], BF16, tag="q2")
                    nc.vector.tensor_tensor(out=q2[:], in0=E_sb[:, p, :], in1=RR[:],
                                            op=OP.mult)

                    # attn^T accumulation: carry over all tokens, then each
                    # S_j over tokens in buckets > j
                    at = psattn.tile([128, CH], F32, tag="attn")
                    for hh in range(2):
                        r0 = hh * 64
                        nc.tensor.matmul(
                            at[r0:r0 + 64, :], c_bf[r0:r0 + 64, p, :],
                            q2[r0:r0 + 64, :], start=True, stop=False,
                            tile_position=(r0, r0))
                        for j in range(UC - 1):
                            q0 = (j + 1) * BUCKET
                            nc.tensor.matmul(
                                at[r0:r0 + 64, q0:CH],
                                s_bf[r0:r0 + 64, j // 2, j % 2, 0:E],
                                q2[r0:r0 + 64, q0:CH], start=False,
                                stop=(j == UC - 2), tile_position=(r0, r0))
                    nc.vector.tensor_copy(out=atn[:, p, :], in_=at[:])

                    # advance the running carry (reads above see the old state)
                    c_red = smallp.tile([128, E], F32, tag="c_red")
                    nc.vector.tensor_reduce(
                        out=c_red[:],
                        in_=s_bf[:].rearrange("p a b e1 -> p e1 (a b)")[:, 0:E, :],
                        axis=mybir.AxisListType.X, op=OP.add)
                    nc.vector.tensor_tensor(out=c_ms[:, p, 0:E], in0=c_ms[:, p, 0:E],
                                            in1=c_red[:], op=OP.add)
                    nc.vector.tensor_copy(out=c_ms[:, p, E:E + 1], in_=ksc[:, UC - 1:UC])
                    nc.vector.tensor_copy(out=c_bf[:, p, :], in_=c_ms[:, p, 0:E])

                # output projection: out_chunk = attn^T.T @ Wo (contract feats)
                osb = outp.tile([128, PAIRS, DIM], BF16, tag="osb")
                for tt in range(PAIRS):
                    for half in range(2):
                        op_ = psproj.tile([128, 512], F32, tag="proj")
                        for p in range(PAIRS):
                            nc.tensor.matmul(
                                op_[:], atn[:, p, tt * 128:(tt + 1) * 128],
                                wo_sb[:, p, half * 512:(half + 1) * 512],
                                start=(p == 0), stop=(p == PAIRS - 1))
                        nc.scalar.activation(
                            out=osb[:, tt, half * 512:(half + 1) * 512],
                            in_=op_[:], func=AF.Copy)
                nc.gpsimd.dma_start(
                    out=out[c * CH:(c + 1) * CH, :].rearrange(
                        "(tt p) d -> p tt d", p=128),
                    in_=osb[:])

            for c in range(n_chunks + 1):
                if c < n_chunks:
                    emit_proj(c)
                if c >= 1:
                    emit_attn(c - 1)

    nc.finalize()
    return nc


def _orp():
    m = np.zeros((128, 128), dtype=ml_dtypes.bfloat16)
    for p in range(PAIRS):
        m[32 * p, 0:64] = 1
        m[32 * p + 1, 64:128] = 1
    return m


def build_in_maps(x, Wq, Wk, Wv, Wo):
    bf = ml_dtypes.bfloat16
    x = np.asarray(x)
    Wq, Wk, Wv, Wo = (np.asarray(w) for w in (Wq, Wk, Wv, Wo))
    in_maps = []
    for c in range(8):
        b, g = c // 2, c % 2
        sl = slice(g * F, (g + 1) * F)
        in_maps.append({
            "x": np.ascontiguousarray(x[b].T).astype(bf),
            "wq": Wq[:, sl].astype(bf),
            "wk": Wk[:, sl].astype(bf),
            "wv": Wv[:, sl].astype(bf),
            "wo": Wo[sl, :].astype(bf),
            "orp": _orp(),
        })
    return in_maps


def kernel(x, Wq, Wk, Wv, Wo, bo):
    from concourse.bass_utils import run_bass_kernel_spmd

    if "nc" not in _NC_CACHE:
        _NC_CACHE["nc"] = build_nc()
    nc = _NC_CACHE["nc"]

    in_maps = build_in_maps(x, Wq, Wk, Wv, Wo)
    res = run_bass_kernel_spmd(nc, in_maps, core_ids=list(range(8)))
    outs = [res.results[c]["out"].astype(np.float32) for c in range(8)]
    full = np.stack([outs[2 * b] + outs[2 * b + 1] for b in range(B)], axis=0)
    return (full + np.asarray(bo)[None, None, :].astype(np.float32)).astype(np.float32)
